# revision 11
# baseline (speedup 1.0000x reference)
"""TRN2 Bass kernel for nn_CosClassifier: sim = 10*scalar * cos_sim(inputs, proto).

Data-parallel over 8 NeuronCores: each core computes a (2048, 4096) slab of the
(16384, 4096) similarity matrix. Per core:
  1. Inputs land as 512KB subgroups split across BOTH HWDGE queues (sync +
     scalar) so x and proto load concurrently and compute starts ~9us in.
  2. proto subgroups: row norms (ACT Square+accum -> Sqrt, DVE reciprocal,
     * scalar), rows scaled in place. x subgroups: norms only; the 10/||x||
     row scale is folded into the PSUM drains.
  3. PE-transposes 128x128-blockwise, 4 per PSUM bank; one 512-wide
     PSUM->SBUF copy per bank casts to float32r (TF32-like) so the main
     matmul runs at 1 cycle/row.
  4. dots matmul in fp32r (k-alternating lhsT; same-lhsT b2b fp32r is
     pathologically slow), fp32 accumulate in PSUM; drains apply the x row
     scale (ACT Identity-with-scale / DVE tensor_scalar) in two phases of
     4 n-blocks so 1MB contiguous output DMAs flow from ~20us.
"""
import sys

sys.path.insert(0, "/opt/trn_rl_repo")

import numpy as np

B, C, D = 16384, 4096, 256
NCORES = 8
BS = B // NCORES          # 2048 rows per core
NB = BS // 128            # 16 b-tiles per core
NCT = C // 128            # 32 c-tiles (proto rows)
NK = D // 128             # 2 k-tiles
NN = C // 512             # 8 n-blocks of 512
SGT = 4                   # tiles per subgroup (512KB)
XSG = NB // SGT           # 4 x subgroups
PSG = NCT // SGT          # 8 proto subgroups
NPH = 2                   # output phases (4 n-blocks each)

_compiled = None


def _build():
    import concourse.bacc as bacc
    import concourse.mybir as mybir
    import concourse.tile as tile

    f32 = mybir.dt.float32
    f32r = mybir.dt.float32r
    Act = mybir.ActivationFunctionType

    nc = bacc.Bacc("TRN2", target_bir_lowering=False, debug=False,
                   num_devices=NCORES)

    x_d = nc.dram_tensor("x", [BS, D], f32, kind="ExternalInput").ap()
    p_d = nc.dram_tensor("proto", [C, D], f32, kind="ExternalInput").ap()
    s_d = nc.dram_tensor("scalar", [1, 1], f32, kind="ExternalInput").ap()
    id_d = nc.dram_tensor("identity", [128, 128], f32, kind="ExternalInput").ap()
    out_d = nc.dram_tensor("out", [BS, C], f32, kind="ExternalOutput").ap()

    with tile.TileContext(nc) as tc:
        with tc.tile_pool(name="sbuf", bufs=1) as pool, \
             tc.tile_pool(name="outp", bufs=4) as outp, \
             tc.tile_pool(name="psum_t", bufs=2, space="PSUM") as psum_t, \
             tc.tile_pool(name="psum_m", bufs=6, space="PSUM") as psum_m:

            x_r = x_d.rearrange("(n p) d -> p n d", p=128)       # [128, NB, 256]
            p_r = p_d.rearrange("(n p) d -> p n d", p=128)       # [128, NCT, 256]

            def load_x(g, eng):
                t = pool.tile([128, SGT * D], f32, tag=f"xsg{g}")
                eng.dma_start(
                    t[:].rearrange("p (n d) -> p n d", d=D),
                    x_r[:, g * SGT:(g + 1) * SGT, :])
                return t

            def load_p(g, eng):
                t = pool.tile([128, SGT * D], f32, tag=f"psg{g}")
                eng.dma_start(
                    t[:].rearrange("p (n d) -> p n d", d=D),
                    p_r[:, g * SGT:(g + 1) * SGT, :])
                return t

            xsg = {}
            psg = {}
            # two HWDGE queues load concurrently
            ident = pool.tile([128, 128], f32, tag="ident")
            nc.sync.dma_start(ident[:], id_d[:, :])
            sc = pool.tile([1, 1], f32, tag="sc")
            nc.sync.dma_start(sc[:], s_d[:, :])
            psg[0] = load_p(0, nc.scalar)
            xsg[0] = load_x(0, nc.sync)
            xsg[1] = load_x(1, nc.scalar)
            psg[1] = load_p(1, nc.sync)
            psg[2] = load_p(2, nc.scalar)
            xsg[2] = load_x(2, nc.sync)
            xsg[3] = load_x(3, nc.scalar)
            psg[3] = load_p(3, nc.sync)
            psg[4] = load_p(4, nc.scalar)
            psg[5] = load_p(5, nc.sync)
            psg[6] = load_p(6, nc.scalar)
            psg[7] = load_p(7, nc.sync)

            sc_b = pool.tile([128, 1], f32, tag="sc_b")
            nc.gpsimd.partition_broadcast(sc_b[:], sc[:])

            # transposed operands (f32r)
            # xt: k-block k at cols k*BS, b-tile i at +i*128
            xt = pool.tile([128, NK * BS], f32r, tag="xt")
            # pt: k-block k at cols k*C, c-tile j at +j*128
            pt = pool.tile([128, NK * C], f32r, tag="pt")

            # per-b-tile 10*scalar/||x|| row scales, applied at drain time
            invx = []
            for i in range(NB):
                iv = pool.tile([128, 1], f32, tag=f"invx{i}")
                invx.append(iv)

            cast_flip = [0]

            def process_subgroup(grp, gi, is_x, dst, dst_stride):
                for t in range(SGT):
                    src = grp[:, t * D:(t + 1) * D]
                    ssq = pool.tile([128, 1], f32, tag=f"ssq{t % 2}")
                    sq_scr = pool.tile([128, D], f32, tag=f"sqscr{t % 2}")
                    nc.scalar.activation(sq_scr[:], src, Act.Square,
                                         accum_out=ssq[:])
                    nrm = pool.tile([128, 1], f32, tag=f"nrm{t % 2}")
                    # x: sqrt(ssq)/10 (folds the *10); proto: plain norm
                    nc.scalar.activation(nrm[:], ssq[:], Act.Sqrt,
                                         scale=0.01 if is_x else 1.0)
                    if is_x:
                        inv = invx[gi * SGT + t]
                        nc.vector.reciprocal(inv[:], nrm[:])
                        nc.vector.tensor_mul(inv[:], inv[:], sc_b[:])
                    else:
                        inv = pool.tile([128, 1], f32, tag=f"inv{t % 2}")
                        nc.vector.reciprocal(inv[:], nrm[:])
                        nc.vector.tensor_scalar_mul(src, src, inv[:])
                # 4 transposes share one PSUM bank; one 512-wide cast drains it
                for k in range(NK):
                    tp = psum_t.tile([128, SGT * 128], f32, tag="tp")
                    for t in range(SGT):
                        nc.tensor.transpose(
                            tp[:, t * 128:(t + 1) * 128],
                            grp[:, t * D + k * 128: t * D + (k + 1) * 128],
                            ident[:])
                    cdst = dst[:, k * dst_stride + gi * SGT * 128:
                               k * dst_stride + (gi + 1) * SGT * 128]
                    if cast_flip[0] % 2 == 0:
                        nc.scalar.copy(cdst, tp[:])
                    else:
                        nc.vector.tensor_copy(cdst, tp[:])
                    cast_flip[0] += 1

            # process in DMA arrival order
            process_subgroup(xsg[0], 0, True, xt, BS)
            process_subgroup(psg[0], 0, False, pt, C)
            process_subgroup(xsg[1], 1, True, xt, BS)
            process_subgroup(psg[1], 1, False, pt, C)
            process_subgroup(psg[2], 2, False, pt, C)
            process_subgroup(xsg[2], 2, True, xt, BS)
            process_subgroup(xsg[3], 3, True, xt, BS)
            process_subgroup(psg[3], 3, False, pt, C)
            for g in range(4, PSG):
                process_subgroup(psg[g], g, False, pt, C)

            # ---- main matmul + drain (x row scale folded into drains) ----
            drain_flip = [0]
            for h in range(NPH):
                for i in range(NB):
                    oq = outp.tile([128, (NN // NPH) * 512], f32, tag="oq")
                    for nn_ in range(NN // NPH):
                        n = (NN // NPH) * h + nn_
                        ps = psum_m.tile([128, 512], f32, tag="mm")
                        for k in range(NK):
                            nc.tensor.matmul(
                                ps[:],
                                xt[:, k * BS + i * 128: k * BS + (i + 1) * 128],
                                pt[:, k * C + n * 512: k * C + (n + 1) * 512],
                                start=(k == 0), stop=(k == NK - 1))
                        dst = oq[:, nn_ * 512:(nn_ + 1) * 512]
                        # drains: 7 of 16 on ACT, 9 of 16 on DVE
                        if (drain_flip[0] * 7) % 16 < 7:
                            nc.scalar.activation(dst, ps[:], Act.Identity,
                                                 scale=invx[i][:])
                        else:
                            nc.vector.tensor_scalar_mul(dst, ps[:], invx[i][:])
                        drain_flip[0] += 1
                    nc.sync.dma_start(
                        out_d[i * 128:(i + 1) * 128,
                              h * (C // NPH):(h + 1) * (C // NPH)], oq[:])

    nc.compile()
    return nc


def _get_compiled():
    global _compiled
    if _compiled is None:
        _compiled = _build()
    return _compiled


def kernel(inputs, proto, scalar, _trace=False, **_tr_kw):
    from concourse.bass_utils import run_bass_kernel_spmd

    nc = _get_compiled()
    inputs = np.ascontiguousarray(inputs, dtype=np.float32)
    proto = np.ascontiguousarray(proto, dtype=np.float32)
    sc = np.asarray(scalar, dtype=np.float32).reshape(1, 1)
    ident = np.eye(128, dtype=np.float32)

    in_maps = []
    for c in range(NCORES):
        in_maps.append({
            "x": inputs[c * BS:(c + 1) * BS],
            "proto": proto,
            "scalar": sc,
            "identity": ident,
        })
    res = run_bass_kernel_spmd(nc, in_maps, core_ids=list(range(NCORES)),
                               trace=_trace, **_tr_kw)
    out = np.concatenate([res.results[c]["out"] for c in range(NCORES)], axis=0)
    if _trace:
        kernel.last_results = res
    return out


# revision 12
# speedup vs baseline: 1.1806x; 1.1806x over previous
"""TRN2 Bass kernel for nn_CosClassifier: sim = 10*scalar * cos_sim(inputs, proto).

Data-parallel over 8 NeuronCores: each core computes a (2048, 4096) slab of the
(16384, 4096) similarity matrix. Per core:
  1. DMA in x-slab (2048,256) in 4x512KB subgroups, proto (4096,256) in
     8x512KB subgroups, interleaved so compute starts early.
  2. Per subgroup as it lands: row norms (ACT Square+accum -> Sqrt, DVE
     reciprocal), row scaling (x by 10/||x||, proto by scalar/||p||), then
     PE-transposes 128x128-blockwise, 4 per PSUM bank; one 512-wide
     PSUM->SBUF copy per bank casts to float32r (TF32-like) so the main
     matmul runs at 1 cycle/row.
  3. dots matmul in fp32r (k-alternating lhsT; same-lhsT b2b fp32r is
     pathologically slow), fp32 accumulate in PSUM; plain-copy drains split
     ACT/DVE in four phases of n-block pairs so 512KB contiguous output DMAs
     flow from ~20us.
"""
import sys

sys.path.insert(0, "/opt/trn_rl_repo")

import numpy as np

B, C, D = 16384, 4096, 256
NCORES = 8
BS = B // NCORES          # 2048 rows per core
NB = BS // 128            # 16 b-tiles per core
NCT = C // 128            # 32 c-tiles (proto rows)
NK = D // 128             # 2 k-tiles
NN = C // 512             # 8 n-blocks of 512
SGT = 4                   # tiles per subgroup (512KB)
XSG = NB // SGT           # 4 x subgroups
PSG = NCT // SGT          # 8 proto subgroups

_compiled = None


def _build():
    import concourse.bacc as bacc
    import concourse.mybir as mybir
    import concourse.tile as tile

    f32 = mybir.dt.float32
    f32r = mybir.dt.float32r
    Act = mybir.ActivationFunctionType

    nc = bacc.Bacc("TRN2", target_bir_lowering=False, debug=False,
                   num_devices=NCORES)

    x_d = nc.dram_tensor("x", [BS, D], f32, kind="ExternalInput").ap()
    p_d = nc.dram_tensor("proto", [C, D], f32, kind="ExternalInput").ap()
    s_d = nc.dram_tensor("scalar", [1, 1], f32, kind="ExternalInput").ap()
    id_d = nc.dram_tensor("identity", [128, 128], f32, kind="ExternalInput").ap()
    out_d = nc.dram_tensor("out", [BS, C], f32, kind="ExternalOutput").ap()

    with tile.TileContext(nc) as tc:
        with tc.tile_pool(name="sbuf", bufs=1) as pool, \
             tc.tile_pool(name="outp", bufs=6) as outp, \
             tc.tile_pool(name="psum_t", bufs=2, space="PSUM") as psum_t, \
             tc.tile_pool(name="psum_m", bufs=6, space="PSUM") as psum_m:

            x_r = x_d.rearrange("(n p) d -> p n d", p=128)       # [128, NB, 256]
            p_r = p_d.rearrange("(n p) d -> p n d", p=128)       # [128, NCT, 256]

            def load_x(g):
                t = pool.tile([128, SGT * D], f32, tag=f"xsg{g}")
                nc.sync.dma_start(
                    t[:].rearrange("p (n d) -> p n d", d=D),
                    x_r[:, g * SGT:(g + 1) * SGT, :])
                return t

            def load_p(g):
                t = pool.tile([128, SGT * D], f32, tag=f"psg{g}")
                nc.sync.dma_start(
                    t[:].rearrange("p (n d) -> p n d", d=D),
                    p_r[:, g * SGT:(g + 1) * SGT, :])
                return t

            xsg = {}
            psg = {}
            xsg[0] = load_x(0)
            ident = pool.tile([128, 128], f32, tag="ident")
            nc.sync.dma_start(ident[:], id_d[:, :])
            sc = pool.tile([1, 1], f32, tag="sc")
            nc.sync.dma_start(sc[:], s_d[:, :])
            sc_b = pool.tile([128, 1], f32, tag="sc_b")
            nc.gpsimd.partition_broadcast(sc_b[:], sc[:])
            psg[0] = load_p(0)
            psg[1] = load_p(1)
            xsg[1] = load_x(1)
            psg[2] = load_p(2)
            xsg[2] = load_x(2)
            psg[3] = load_p(3)
            xsg[3] = load_x(3)
            for g in range(4, PSG):
                psg[g] = load_p(g)

            # transposed operands (f32r)
            # xt: k-block k at cols k*BS, b-tile i at +i*128
            xt = pool.tile([128, NK * BS], f32r, tag="xt")
            # pt: k-block k at cols k*C, c-tile j at +j*128
            pt = pool.tile([128, NK * C], f32r, tag="pt")

            cast_flip = [0]

            def process_subgroup(grp, gi, with_scalar, dst, dst_stride):
                for t in range(SGT):
                    src = grp[:, t * D:(t + 1) * D]
                    ssq = pool.tile([128, 1], f32, tag=f"ssq{t % 2}")
                    sq_scr = pool.tile([128, D], f32, tag=f"sqscr{t % 2}")
                    nc.scalar.activation(sq_scr[:], src, Act.Square,
                                         accum_out=ssq[:])
                    nrm = pool.tile([128, 1], f32, tag=f"nrm{t % 2}")
                    # x: sqrt(ssq)/10 (folds *10); proto: plain norm
                    nc.scalar.activation(nrm[:], ssq[:], Act.Sqrt,
                                         scale=1.0 if with_scalar else 0.01)
                    inv = pool.tile([128, 1], f32, tag=f"inv{t % 2}")
                    nc.vector.reciprocal(inv[:], nrm[:])
                    if with_scalar:
                        nc.vector.tensor_mul(inv[:], inv[:], sc_b[:])
                    nc.vector.tensor_scalar_mul(src, src, inv[:])
                # 4 transposes share one PSUM bank; one 512-wide cast drains it
                for k in range(NK):
                    tp = psum_t.tile([128, SGT * 128], f32, tag="tp")
                    for t in range(SGT):
                        nc.tensor.transpose(
                            tp[:, t * 128:(t + 1) * 128],
                            grp[:, t * D + k * 128: t * D + (k + 1) * 128],
                            ident[:])
                    cdst = dst[:, k * dst_stride + gi * SGT * 128:
                               k * dst_stride + (gi + 1) * SGT * 128]
                    # casts alternate ACT/DVE
                    if cast_flip[0] % 2 == 0:
                        nc.scalar.copy(cdst, tp[:])
                    else:
                        nc.vector.tensor_copy(cdst, tp[:])
                    cast_flip[0] += 1

            # process in DMA arrival order, x/p interleaved
            process_subgroup(xsg[0], 0, False, xt, BS)
            process_subgroup(psg[0], 0, True, pt, C)
            process_subgroup(psg[1], 1, True, pt, C)
            process_subgroup(xsg[1], 1, False, xt, BS)
            process_subgroup(psg[2], 2, True, pt, C)
            process_subgroup(xsg[2], 2, False, xt, BS)
            process_subgroup(psg[3], 3, True, pt, C)
            process_subgroup(xsg[3], 3, False, xt, BS)
            for g in range(4, PSG):
                process_subgroup(psg[g], g, True, pt, C)

            # ---- main matmul + drain ----
            # phase h covers n-blocks {2h, 2h+1} <-> proto subgroups 2h,2h+1,
            # so MMs start as soon as the matching proto subgroup is ready and
            # 512KB contiguous output DMAs flow from early in the kernel.
            drain_flip = [0]
            for h in range(NN // 2):
                for i in range(NB):
                    oq = outp.tile([128, 1024], f32, tag="oq")
                    for nn_ in range(2):
                        n = 2 * h + nn_
                        ps = psum_m.tile([128, 512], f32, tag="mm")
                        for k in range(NK):
                            nc.tensor.matmul(
                                ps[:],
                                xt[:, k * BS + i * 128: k * BS + (i + 1) * 128],
                                pt[:, k * C + n * 512: k * C + (n + 1) * 512],
                                start=(k == 0), stop=(k == NK - 1))
                        dst = oq[:, nn_ * 512:(nn_ + 1) * 512]
                        # drains: 7 of 16 on ACT, 9 of 16 on DVE
                        if (drain_flip[0] * 7) % 16 < 7:
                            nc.scalar.copy(dst, ps[:])
                        else:
                            nc.vector.tensor_copy(dst, ps[:])
                        drain_flip[0] += 1
                    nc.sync.dma_start(
                        out_d[i * 128:(i + 1) * 128,
                              h * 1024:(h + 1) * 1024], oq[:])

    nc.compile()
    return nc


def _get_compiled():
    global _compiled
    if _compiled is None:
        _compiled = _build()
    return _compiled


def kernel(inputs, proto, scalar, _trace=False, **_tr_kw):
    from concourse.bass_utils import run_bass_kernel_spmd

    nc = _get_compiled()
    inputs = np.ascontiguousarray(inputs, dtype=np.float32)
    proto = np.ascontiguousarray(proto, dtype=np.float32)
    sc = np.asarray(scalar, dtype=np.float32).reshape(1, 1)
    ident = np.eye(128, dtype=np.float32)

    in_maps = []
    for c in range(NCORES):
        in_maps.append({
            "x": inputs[c * BS:(c + 1) * BS],
            "proto": proto,
            "scalar": sc,
            "identity": ident,
        })
    res = run_bass_kernel_spmd(nc, in_maps, core_ids=list(range(NCORES)),
                               trace=_trace, **_tr_kw)
    out = np.concatenate([res.results[c]["out"] for c in range(NCORES)], axis=0)
    if _trace:
        kernel.last_results = res
    return out
